# revision 2
# baseline (speedup 1.0000x reference)
"""AttnPooling Trainium2 kernel v2: 8-core data-parallel, transposed layout.

Host ships x per item PRE-MASKED as xt (128p, 32c, 129) bf16 where
xt[p, c, d] = m_k * x[d, k] for d<128 and m_k at d=128 (masked ones
column), k = c*128+p: tokens on partitions, features on the free axis.
Mask ships as m' = mask/count (bf16), tokens on partitions.

Per item (k = token, 32 chunks of 128 tokens):
  mean_row (1,129)  = sum_c m'_c^T @ xt_c          (PE, accumulating)
  mcol              = transpose(mean_row)           (PE identity transpose)
  v_row (1,128)     = mcol^T @ (Wq^T Wk) + bq^T Wk  (PE + DVE add)
  vb (128,129)      = broadcast v_row, col128=0     (PE 1-partition matmul)
  prod              = xt[:, :, 0:128] * vb          (DVE tensor_tensor, 2x bf16)
  s (128,32)        = segmented sum_d prod          (tree adds: L1 on GPSIMD,
                                                     L2/L3 + 16-wide direct
                                                     tensor_reduce on DVE)
  e (128,32)        = exp(s/sqrt(D))                (Act Exp; no mask fixup --
                                                     masked rows of xt are 0 so
                                                     they add 0 to pooled AND Z)
  pooled_row(1,129) = sum_c e_c^T @ xt_c            (PE; col 128 = Z)
  out_row (1,128)   = pooled_row[0:128] / Z         (DVE recip + tensor_scalar)

Softmax invariance absorbs the 1/c scale in m'. Masked tokens have e=1
but zeroed values/ones-col, so they contribute nothing to pooled or Z.
"""

import sys

sys.path.insert(0, "/opt/trn_rl_repo")

import numpy as np
import ml_dtypes
from contextlib import ExitStack

NI = 16  # items per core
D = 128
K = 4096
NCH = 32  # token chunks per item
W = 129  # chunk width: 128 features + ones column
WS = 130  # stored chunk width (even stride keeps DVE 2x packing)
NCORES = 8
SD = float(1.0 / np.sqrt(128.0))

# chunks whose L1 tree-add runs on GPSIMD instead of DVE (0..NCH)
POOL_L1_CH = 32

_CACHE = {}


def _build():
    import concourse.bass as bass
    from concourse import bacc, mybir, tile

    dt = mybir.dt
    Alu = mybir.AluOpType
    Act = mybir.ActivationFunctionType

    nc = bacc.Bacc(
        "TRN2", target_bir_lowering=False, debug=False, num_devices=NCORES
    )
    xt_d = nc.dram_tensor(
        "xt", [NI, 128, NCH * WS], dt.bfloat16, kind="ExternalInput"
    ).ap()
    mm_d = nc.dram_tensor(
        "mm", [128, NI * NCH], dt.bfloat16, kind="ExternalInput"
    ).ap()
    wq_d = nc.dram_tensor("Wq", [D, D], dt.float32, kind="ExternalInput").ap()
    wk_d = nc.dram_tensor("Wk", [D, D], dt.float32, kind="ExternalInput").ap()
    bq_d = nc.dram_tensor("bq", [D, 1], dt.float32, kind="ExternalInput").ap()
    out_d = nc.dram_tensor("out", [NI, D], dt.float32, kind="ExternalOutput").ap()

    with tile.TileContext(nc) as tc, ExitStack() as ctx:
        per = ctx.enter_context(tc.tile_pool(name="per", bufs=1))
        xp = ctx.enter_context(tc.tile_pool(name="xp", bufs=10))
        prp = ctx.enter_context(tc.tile_pool(name="prp", bufs=4))
        sp = ctx.enter_context(tc.tile_pool(name="sp", bufs=2))
        ep = ctx.enter_context(tc.tile_pool(name="ep", bufs=4))
        trp = ctx.enter_context(tc.tile_pool(name="trp", bufs=3))
        vbbp = ctx.enter_context(tc.tile_pool(name="vbbp", bufs=2))
        rowp = ctx.enter_context(tc.tile_pool(name="rowp", bufs=2))
        colp = ctx.enter_context(tc.tile_pool(name="colp", bufs=2))
        prw = ctx.enter_context(tc.tile_pool(name="prw", bufs=2))
        orp = ctx.enter_context(tc.tile_pool(name="orp", bufs=2))
        # PSUM pools (bank-granular: keep total <= 8)
        mps = ctx.enter_context(tc.tile_pool(name="mps", bufs=2, space="PSUM"))
        mcp = ctx.enter_context(tc.tile_pool(name="mcp", bufs=1, space="PSUM"))
        cps = ctx.enter_context(tc.tile_pool(name="cps", bufs=1, space="PSUM"))
        vps = ctx.enter_context(tc.tile_pool(name="vps", bufs=1, space="PSUM"))
        vbp = ctx.enter_context(tc.tile_pool(name="vbp", bufs=1, space="PSUM"))
        pps = ctx.enter_context(tc.tile_pool(name="pps", bufs=2, space="PSUM"))

        # persistent tiles
        wq = per.tile([D, D], dt.float32, tag="wq")
        wk = per.tile([D, D], dt.float32, tag="wk")
        bqt = per.tile([D, 1], dt.float32, tag="bqt")
        cqkb = per.tile([D, D], dt.bfloat16, tag="cqkb")
        w0b = per.tile([1, D], dt.float32, tag="w0b")
        ones1 = per.tile([1, D], dt.bfloat16, tag="ones1")
        ident1 = per.tile([1, 1], dt.bfloat16, tag="ident1")
        mm = per.tile([128, NI * NCH], dt.bfloat16, tag="mm")
        vss = [
            per.tile([1, W + 1], dt.bfloat16, tag=f"vs{j}", name=f"vs{j}")
            for j in range(3)
        ]

        # ---- setup ----
        nc.sync.dma_start(wq[:, :], wq_d[:, :])
        nc.sync.dma_start(wk[:, :], wk_d[:, :])
        nc.sync.dma_start(bqt[:, :], bq_d[:, :])
        nc.sync.dma_start(mm[:, :], mm_d[:, :])
        nc.vector.memset(ones1[:, :], 1.0)
        nc.vector.memset(ident1[:, :], 1.0)
        for t in vss:
            nc.vector.memset(t[0:1, D : D + 1], 0.0)

        # CQK = Wq^T @ Wk ; w0_row = bq^T @ Wk
        cqk_ps = cps.tile([D, D], dt.float32, tag="c", name="cqk_ps")
        nc.tensor.matmul(cqk_ps[:, :], wq[:, :], wk[:, :], start=True, stop=True)
        nc.scalar.copy(cqkb[:, :], cqk_ps[:, :])
        w0_ps = cps.tile([1, D], dt.float32, tag="c", name="w0_ps")
        nc.tensor.matmul(w0_ps[:, :], bqt[:, :], wk[:, :], start=True, stop=True)
        nc.scalar.copy(w0b[:, :], w0_ps[:, :])

        xts = [None] * NI
        es = [None] * NI
        vbbs = [None] * NI
        prs = [None] * NI
        mrows = [None] * NI
        mcols = [None] * NI
        vpss = [None] * NI
        t1s = [None] * NI
        ss = [None] * NI

        def st_dma(i):
            xt = xp.tile([128, NCH, WS], dt.bfloat16, tag="xt", name=f"xt_{i}")
            xts[i] = xt
            nc.sync.dma_start(
                xt[:, :, :],
                xt_d[i, :, :].rearrange("p (c w) -> p c w", w=WS),
            )

        def st_mean_pe(i):
            xt = xts[i]
            mrow_ps = mps.tile([1, W], dt.float32, tag="m", name=f"mps_{i}")
            for c in range(NCH):
                nc.tensor.matmul(
                    mrow_ps[0:1, :],
                    mm[:, NCH * i + c : NCH * i + c + 1],
                    xt[:, c, 0:W],
                    start=(c == 0),
                    stop=(c == NCH - 1),
                )
            mrows[i] = mrow_ps

        def st_mean_act(i):
            mrowb = rowp.tile([1, D], dt.bfloat16, tag="mr", name=f"mr_{i}")
            nc.scalar.copy(mrowb[:, :], mrows[i][0:1, 0:D])
            mrows[i] = mrowb

        def st_transpose_pe(i):
            mcol_ps = mcp.tile([128, 1], dt.bfloat16, tag="mc", name=f"mcps_{i}")
            nc.tensor.transpose(mcol_ps[:, :], mrows[i][0:1, :], ident1[:, :])
            mcols[i] = mcol_ps

        def st_transpose_act(i):
            mcolb = colp.tile([128, 1], dt.bfloat16, tag="mcb", name=f"mcb_{i}")
            nc.scalar.copy(mcolb[:, :], mcols[i][:, :])
            mcols[i] = mcolb

        def st_vps_pe(i):
            v_ps = vps.tile([1, D], dt.float32, tag="v", name=f"vps_{i}")
            nc.tensor.matmul(
                v_ps[0:1, :], mcols[i][:, :], cqkb[:, :], start=True, stop=True
            )
            vpss[i] = v_ps

        def st_vs_dve(i):
            vs = vss[i % 3]
            nc.vector.tensor_tensor(
                vs[0:1, 0:D], vpss[i][0:1, :], w0b[0:1, :], op=Alu.add
            )
            vpss[i] = vs

        def st_vb_pe(i):
            vb_ps = vbp.tile([128, W], dt.float32, tag="vb", name=f"vbps_{i}")
            nc.tensor.matmul(
                vb_ps[:, :], ones1[0:1, :], vpss[i][0:1, 0:W], start=True, stop=True
            )
            vbbs[i] = vb_ps

        def st_vb_act(i):
            vbb = vbbp.tile([128, W], dt.bfloat16, tag="vbb", name=f"vbb_{i}")
            nc.scalar.copy(vbb[:, :], vbbs[i][:, :])
            vbbs[i] = vbb

        def st_products_dve(i):
            xt = xts[i]
            vbb = vbbs[i]
            prod = prp.tile([128, NCH, D + 2], dt.bfloat16, tag="pr", name=f"pr_{i}")
            nc.vector.tensor_tensor(
                prod[:, :, 0:D],
                xt[:, :, 0:D],
                vbb[:, 0:D].unsqueeze(1).broadcast_to((128, NCH, D)),
                op=Alu.mult,
            )
            return prod

        def st_tree_pool(i, prod):
            # L1+L2 on GPSIMD: 128 -> 64 -> 32
            t1 = trp.tile([128, NCH, 66], dt.bfloat16, tag="t1", name=f"t1_{i}")
            t2 = trp.tile([128, NCH, 34], dt.bfloat16, tag="t2", name=f"t2_{i}")
            nc.gpsimd.tensor_tensor(
                t1[:, :, 0:64],
                prod[:, :, 0:64],
                prod[:, :, 64:128],
                op=Alu.add,
            )
            nc.gpsimd.tensor_tensor(
                t2[:, :, 0:32], t1[:, :, 0:32], t1[:, :, 32:64], op=Alu.add
            )
            t1s[i] = (t1, t2)

        def st_tail_dve(i):
            t1, t2 = t1s[i]
            nc.vector.tensor_tensor(
                t1[:, :, 0:16], t2[:, :, 0:16], t2[:, :, 16:32], op=Alu.add
            )
            s = sp.tile([128, NCH], dt.float32, tag="s", name=f"s_{i}")
            nc.vector.tensor_reduce(
                s[:, :], t1[:, :, 0:16], axis=mybir.AxisListType.X, op=Alu.add
            )
            ss[i] = s

        def st_exp_act(i):
            e = ep.tile([128, NCH], dt.bfloat16, tag="e", name=f"e_{i}")
            nc.scalar.activation(e[:, :], ss[i][:, :], Act.Exp, scale=SD)
            es[i] = e

        def st_pooled_pe(i):
            xt = xts[i]
            e = es[i]
            p_ps = pps.tile([1, W], dt.float32, tag="p", name=f"pps_{i}")
            for c in range(NCH):
                nc.tensor.matmul(
                    p_ps[0:1, :],
                    e[:, c : c + 1],
                    xt[:, c, 0:W],
                    start=(c == 0),
                    stop=(c == NCH - 1),
                )
            prs[i] = p_ps

        def st_final(i):
            pr = prw.tile([1, W + 1], dt.float32, tag="prw", name=f"prw_{i}")
            nc.scalar.copy(pr[0:1, 0:W], prs[i][0:1, :])
            nc.vector.reciprocal(pr[0:1, W : W + 1], pr[0:1, D : D + 1])
            orow = orp.tile([1, D], dt.float32, tag="or", name=f"or_{i}")
            nc.scalar.activation(
                orow[0:1, :],
                pr[0:1, 0:D],
                Act.Copy,
                scale=pr[0:1, W : W + 1],
            )
            nc.sync.dma_start(out_d[i : i + 1, :], orow[0:1, :])

        # Staged software pipeline. Stage offsets (item i runs stage s at
        # slot i+off[s]): in steady state every PE op's cross-engine
        # dependency is produced a full slot earlier, so the PE stream
        # never stalls and keeps its p-state ramp. The first items use
        # compressed offsets (brief fill-time stalls beat idle engines).
        OFF_STD = dict(
            dma=0, mean=1, tr=2, vps=3, vb=4, prod=5, tail=6, pooled=7
        )
        OFF_BY_ITEM = {}
        # within-slot emission priority (defines per-engine program order):
        # PE tiny ops lead, then big blocks; DVE big ops, then tail ops
        PRIO = dict(
            tr=0, vps=1, vb=2, pooled=3, mean=4, dma=5, prod=6, tail=7,
            vs=8, tree=9, mean_act=10, tr_act=11, vb_act=12, exp=13, final=14
        )
        STAGE_SLOT = dict(
            dma="dma", mean="mean", mean_act="mean", tr="tr", tr_act="tr",
            vps="vps", vs="vps", vb="vb", vb_act="vb", prod="prod",
            tree="prod", tail="tail", exp="tail", pooled="pooled",
            final="pooled",
        )
        FNS = dict(
            dma=st_dma, mean=st_mean_pe, mean_act=st_mean_act,
            tr=st_transpose_pe, tr_act=st_transpose_act, vps=st_vps_pe,
            vs=st_vs_dve, vb=st_vb_pe, vb_act=st_vb_act, tail=st_tail_dve,
            exp=st_exp_act, pooled=st_pooled_pe, final=st_final,
        )
        # compressed items chain within a slot: emit their stages in
        # dependency order AFTER all steady-state work of that slot
        CHAIN_POS = dict(
            dma=20, mean=21, mean_act=22, tr=23, tr_act=24, vps=25, vs=26,
            vb=27, vb_act=28, prod=29, tree=30, tail=31, exp=32, pooled=33,
            final=34,
        )
        events = []
        for i in range(NI):
            off = OFF_BY_ITEM.get(i)
            key = PRIO if off is None else CHAIN_POS
            off = off or OFF_STD
            for st in PRIO:
                events.append((i + off[STAGE_SLOT[st]], key[st], i, st))
        events.sort()
        prods = {}
        for slot, _, i, st in events:
            if st == "prod":
                prods[i] = st_products_dve(i)
            elif st == "tree":
                st_tree_pool(i, prods[i])
            else:
                FNS[st](i)

    nc.compile()
    return nc


def _get_nc():
    if "nc" not in _CACHE:
        _CACHE["nc"] = _build()
    return _CACHE["nc"]


def _prep_inputs(x, mask, Wq, bq, Wk):
    """Host-side: transposed bf16 layout with ones column + scaled masks."""
    B, N, d, H, Wd = x.shape
    BN = B * N
    xr = np.asarray(x, dtype=np.float32).reshape(BN, d, NCH, 128)
    mr = np.asarray(mask).reshape(BN, NCH, 128).astype(np.float32)
    mt = mr.transpose(0, 2, 1)  # (BN, 128p, NCH)

    xt = np.zeros((BN, 128, NCH, WS), dtype=ml_dtypes.bfloat16)
    # pre-masked values and masked ones column: masked tokens are all-zero
    # rows, so they contribute nothing to pooled sums or Z
    xt[..., 0:d] = xr.transpose(0, 3, 2, 1) * mt[..., None]
    xt[..., d] = mt
    xt = np.ascontiguousarray(xt.reshape(BN, 128, NCH * WS))

    cnt = np.maximum(mr.reshape(BN, -1).sum(axis=1), 1.0)
    mprime = (mt / cnt[:, None, None]).astype(ml_dtypes.bfloat16)

    wqc = np.ascontiguousarray(np.asarray(Wq, dtype=np.float32))
    wkc = np.ascontiguousarray(np.asarray(Wk, dtype=np.float32))
    bq2 = np.ascontiguousarray(np.asarray(bq, dtype=np.float32).reshape(d, 1))

    in_maps = []
    for cid in range(NCORES):
        sl = slice(cid * NI, (cid + 1) * NI)
        mm = np.ascontiguousarray(
            mprime[sl].transpose(1, 0, 2).reshape(128, NI * NCH)
        )
        in_maps.append(
            {
                "xt": np.ascontiguousarray(xt[sl]),
                "mm": mm,
                "Wq": wqc,
                "Wk": wkc,
                "bq": bq2,
            }
        )
    return in_maps


def kernel(x, mask, Wq, bq, Wk, bk):
    from concourse.bass_utils import run_bass_kernel_spmd

    nc = _get_nc()
    B, N, d, H, Wd = x.shape
    in_maps = _prep_inputs(x, mask, Wq, bq, Wk)
    res = run_bass_kernel_spmd(nc, in_maps, core_ids=list(range(NCORES)))
    parts = [np.asarray(res.results[c]["out"]) for c in range(NCORES)]
    return np.concatenate(parts, axis=0).reshape(B, N, d).astype(np.float32)


# revision 3
# speedup vs baseline: 1.0048x; 1.0048x over previous
"""AttnPooling Trainium2 kernel v2: 8-core data-parallel, transposed layout.

Host ships x per item PRE-MASKED as xt (128p, 32c, 129) bf16 where
xt[p, c, d] = m_k * x[d, k] for d<128 and m_k at d=128 (masked ones
column), k = c*128+p: tokens on partitions, features on the free axis.
Mask ships as m' = mask/count (bf16), tokens on partitions.

Per item (k = token, 32 chunks of 128 tokens):
  mean_row (1,129)  = sum_c m'_c^T @ xt_c          (PE, accumulating)
  mcol              = transpose(mean_row)           (PE identity transpose)
  v_row (1,128)     = mcol^T @ (Wq^T Wk) + bq^T Wk  (PE + DVE add)
  vb (128,129)      = broadcast v_row, col128=0     (PE 1-partition matmul)
  prod              = xt[:, :, 0:128] * vb          (DVE tensor_tensor, 2x bf16)
  s (128,32)        = segmented sum_d prod          (tree adds: L1 on GPSIMD,
                                                     L2/L3 + 16-wide direct
                                                     tensor_reduce on DVE)
  e (128,32)        = exp(s/sqrt(D))                (Act Exp; no mask fixup --
                                                     masked rows of xt are 0 so
                                                     they add 0 to pooled AND Z)
  pooled_row(1,129) = sum_c e_c^T @ xt_c            (PE; col 128 = Z)
  out_row (1,128)   = pooled_row[0:128] / Z         (DVE recip + tensor_scalar)

Softmax invariance absorbs the 1/c scale in m'. Masked tokens have e=1
but zeroed values/ones-col, so they contribute nothing to pooled or Z.
"""

import sys

sys.path.insert(0, "/opt/trn_rl_repo")

import numpy as np
import ml_dtypes
from contextlib import ExitStack

NI = 16  # items per core
D = 128
K = 4096
NCH = 32  # token chunks per item
W = 129  # chunk width: 128 features + ones column
WS = 129  # stored chunk width
NCORES = 8
SD = float(1.0 / np.sqrt(128.0))

# chunks whose L1 tree-add runs on GPSIMD instead of DVE (0..NCH)
POOL_L1_CH = 32

_CACHE = {}


def _build():
    import concourse.bass as bass
    from concourse import bacc, mybir, tile

    dt = mybir.dt
    Alu = mybir.AluOpType
    Act = mybir.ActivationFunctionType

    nc = bacc.Bacc(
        "TRN2", target_bir_lowering=False, debug=False, num_devices=NCORES
    )
    xt_d = nc.dram_tensor(
        "xt", [NI, 128, NCH * WS], dt.bfloat16, kind="ExternalInput"
    ).ap()
    mm_d = nc.dram_tensor(
        "mm", [128, NI * NCH], dt.bfloat16, kind="ExternalInput"
    ).ap()
    wq_d = nc.dram_tensor("Wq", [D, D], dt.float32, kind="ExternalInput").ap()
    wk_d = nc.dram_tensor("Wk", [D, D], dt.float32, kind="ExternalInput").ap()
    bq_d = nc.dram_tensor("bq", [D, 1], dt.float32, kind="ExternalInput").ap()
    out_d = nc.dram_tensor("out", [NI, D], dt.float32, kind="ExternalOutput").ap()

    with tile.TileContext(nc) as tc, ExitStack() as ctx:
        per = ctx.enter_context(tc.tile_pool(name="per", bufs=1))
        xp = ctx.enter_context(tc.tile_pool(name="xp", bufs=10))
        prp = ctx.enter_context(tc.tile_pool(name="prp", bufs=4))
        sp = ctx.enter_context(tc.tile_pool(name="sp", bufs=2))
        ep = ctx.enter_context(tc.tile_pool(name="ep", bufs=4))
        trp = ctx.enter_context(tc.tile_pool(name="trp", bufs=3))
        vbbp = ctx.enter_context(tc.tile_pool(name="vbbp", bufs=2))
        rowp = ctx.enter_context(tc.tile_pool(name="rowp", bufs=2))
        colp = ctx.enter_context(tc.tile_pool(name="colp", bufs=2))
        prw = ctx.enter_context(tc.tile_pool(name="prw", bufs=2))
        orp = ctx.enter_context(tc.tile_pool(name="orp", bufs=2))
        # PSUM pools (bank-granular: keep total <= 8)
        mps = ctx.enter_context(tc.tile_pool(name="mps", bufs=2, space="PSUM"))
        mcp = ctx.enter_context(tc.tile_pool(name="mcp", bufs=1, space="PSUM"))
        cps = ctx.enter_context(tc.tile_pool(name="cps", bufs=1, space="PSUM"))
        vps = ctx.enter_context(tc.tile_pool(name="vps", bufs=1, space="PSUM"))
        vbp = ctx.enter_context(tc.tile_pool(name="vbp", bufs=1, space="PSUM"))
        pps = ctx.enter_context(tc.tile_pool(name="pps", bufs=2, space="PSUM"))

        # persistent tiles
        wq = per.tile([D, D], dt.float32, tag="wq")
        wk = per.tile([D, D], dt.float32, tag="wk")
        bqt = per.tile([D, 1], dt.float32, tag="bqt")
        cqkb = per.tile([D, D], dt.bfloat16, tag="cqkb")
        w0b = per.tile([1, D], dt.float32, tag="w0b")
        ones1 = per.tile([1, D], dt.bfloat16, tag="ones1")
        ident1 = per.tile([1, 1], dt.bfloat16, tag="ident1")
        mm = per.tile([128, NI * NCH], dt.bfloat16, tag="mm")
        vss = [
            per.tile([1, W + 1], dt.bfloat16, tag=f"vs{j}", name=f"vs{j}")
            for j in range(3)
        ]

        # ---- setup ----
        nc.sync.dma_start(wq[:, :], wq_d[:, :])
        nc.sync.dma_start(wk[:, :], wk_d[:, :])
        nc.sync.dma_start(bqt[:, :], bq_d[:, :])
        nc.sync.dma_start(mm[:, :], mm_d[:, :])
        nc.vector.memset(ones1[:, :], 1.0)
        nc.vector.memset(ident1[:, :], 1.0)
        for t in vss:
            nc.vector.memset(t[0:1, D : D + 1], 0.0)

        # CQK = Wq^T @ Wk ; w0_row = bq^T @ Wk
        cqk_ps = cps.tile([D, D], dt.float32, tag="c", name="cqk_ps")
        nc.tensor.matmul(cqk_ps[:, :], wq[:, :], wk[:, :], start=True, stop=True)
        nc.scalar.copy(cqkb[:, :], cqk_ps[:, :])
        w0_ps = cps.tile([1, D], dt.float32, tag="c", name="w0_ps")
        nc.tensor.matmul(w0_ps[:, :], bqt[:, :], wk[:, :], start=True, stop=True)
        nc.scalar.copy(w0b[:, :], w0_ps[:, :])

        xts = [None] * NI
        es = [None] * NI
        vbbs = [None] * NI
        prs = [None] * NI
        mrows = [None] * NI
        mcols = [None] * NI
        vpss = [None] * NI
        t1s = [None] * NI
        ss = [None] * NI

        def st_dma(i):
            xt = xp.tile([128, NCH, WS], dt.bfloat16, tag="xt", name=f"xt_{i}")
            xts[i] = xt
            nc.sync.dma_start(
                xt[:, :, :],
                xt_d[i, :, :].rearrange("p (c w) -> p c w", w=WS),
            )

        def st_mean_pe(i):
            xt = xts[i]
            mrow_ps = mps.tile([1, W], dt.float32, tag="m", name=f"mps_{i}")
            for c in range(NCH):
                nc.tensor.matmul(
                    mrow_ps[0:1, :],
                    mm[:, NCH * i + c : NCH * i + c + 1],
                    xt[:, c, 0:W],
                    start=(c == 0),
                    stop=(c == NCH - 1),
                )
            mrows[i] = mrow_ps

        def st_mean_act(i):
            mrowb = rowp.tile([1, D], dt.bfloat16, tag="mr", name=f"mr_{i}")
            nc.scalar.copy(mrowb[:, :], mrows[i][0:1, 0:D])
            mrows[i] = mrowb

        def st_transpose_pe(i):
            mcol_ps = mcp.tile([128, 1], dt.bfloat16, tag="mc", name=f"mcps_{i}")
            nc.tensor.transpose(mcol_ps[:, :], mrows[i][0:1, :], ident1[:, :])
            mcols[i] = mcol_ps

        def st_transpose_act(i):
            mcolb = colp.tile([128, 1], dt.bfloat16, tag="mcb", name=f"mcb_{i}")
            nc.scalar.copy(mcolb[:, :], mcols[i][:, :])
            mcols[i] = mcolb

        def st_vps_pe(i):
            v_ps = vps.tile([1, D], dt.float32, tag="v", name=f"vps_{i}")
            nc.tensor.matmul(
                v_ps[0:1, :], mcols[i][:, :], cqkb[:, :], start=True, stop=True
            )
            vpss[i] = v_ps

        def st_vs_dve(i):
            vs = vss[i % 3]
            nc.vector.tensor_tensor(
                vs[0:1, 0:D], vpss[i][0:1, :], w0b[0:1, :], op=Alu.add
            )
            vpss[i] = vs

        def st_vb_pe(i):
            vb_ps = vbp.tile([128, W], dt.float32, tag="vb", name=f"vbps_{i}")
            nc.tensor.matmul(
                vb_ps[:, :], ones1[0:1, :], vpss[i][0:1, 0:W], start=True, stop=True
            )
            vbbs[i] = vb_ps

        def st_vb_act(i):
            vbb = vbbp.tile([128, W], dt.bfloat16, tag="vbb", name=f"vbb_{i}")
            nc.scalar.copy(vbb[:, :], vbbs[i][:, :])
            vbbs[i] = vbb

        def st_products_dve(i):
            xt = xts[i]
            vbb = vbbs[i]
            prod = prp.tile([128, NCH, D + 2], dt.bfloat16, tag="pr", name=f"pr_{i}")
            nc.vector.tensor_tensor(
                prod[:, :, 0:D],
                xt[:, :, 0:D],
                vbb[:, 0:D].unsqueeze(1).broadcast_to((128, NCH, D)),
                op=Alu.mult,
            )
            return prod

        def st_tree_pool(i, prod):
            # L1+L2 on GPSIMD: 128 -> 64 -> 32
            t1 = trp.tile([128, NCH, 66], dt.bfloat16, tag="t1", name=f"t1_{i}")
            t2 = trp.tile([128, NCH, 34], dt.bfloat16, tag="t2", name=f"t2_{i}")
            nc.gpsimd.tensor_tensor(
                t1[:, :, 0:64],
                prod[:, :, 0:64],
                prod[:, :, 64:128],
                op=Alu.add,
            )
            nc.gpsimd.tensor_tensor(
                t2[:, :, 0:32], t1[:, :, 0:32], t1[:, :, 32:64], op=Alu.add
            )
            t1s[i] = (t1, t2)

        def st_tail_dve(i):
            t1, t2 = t1s[i]
            nc.vector.tensor_tensor(
                t1[:, :, 0:16], t2[:, :, 0:16], t2[:, :, 16:32], op=Alu.add
            )
            s = sp.tile([128, NCH], dt.float32, tag="s", name=f"s_{i}")
            nc.vector.tensor_reduce(
                s[:, :], t1[:, :, 0:16], axis=mybir.AxisListType.X, op=Alu.add
            )
            ss[i] = s

        def st_exp_act(i):
            e = ep.tile([128, NCH], dt.bfloat16, tag="e", name=f"e_{i}")
            nc.scalar.activation(e[:, :], ss[i][:, :], Act.Exp, scale=SD)
            es[i] = e

        def st_pooled_pe(i):
            xt = xts[i]
            e = es[i]
            p_ps = pps.tile([1, W], dt.float32, tag="p", name=f"pps_{i}")
            for c in range(NCH):
                nc.tensor.matmul(
                    p_ps[0:1, :],
                    e[:, c : c + 1],
                    xt[:, c, 0:W],
                    start=(c == 0),
                    stop=(c == NCH - 1),
                )
            prs[i] = p_ps

        def st_final(i):
            pr = prw.tile([1, W + 1], dt.float32, tag="prw", name=f"prw_{i}")
            nc.scalar.copy(pr[0:1, 0:W], prs[i][0:1, :])
            nc.vector.reciprocal(pr[0:1, W : W + 1], pr[0:1, D : D + 1])
            orow = orp.tile([1, D], dt.float32, tag="or", name=f"or_{i}")
            nc.scalar.activation(
                orow[0:1, :],
                pr[0:1, 0:D],
                Act.Copy,
                scale=pr[0:1, W : W + 1],
            )
            nc.sync.dma_start(out_d[i : i + 1, :], orow[0:1, :])

        # Staged software pipeline. Stage offsets (item i runs stage s at
        # slot i+off[s]): in steady state every PE op's cross-engine
        # dependency is produced a full slot earlier, so the PE stream
        # never stalls and keeps its p-state ramp. The first items use
        # compressed offsets (brief fill-time stalls beat idle engines).
        OFF_STD = dict(
            dma=0, mean=1, tr=2, vps=3, vb=4, prod=5, tail=6, pooled=7
        )
        OFF_BY_ITEM = {}
        # within-slot emission priority (defines per-engine program order):
        # PE tiny ops lead, then big blocks; DVE big ops, then tail ops
        PRIO = dict(
            tr=0, vps=1, vb=2, pooled=3, mean=4, dma=5, prod=6, tail=7,
            vs=8, tree=9, mean_act=10, tr_act=11, vb_act=12, exp=13, final=14
        )
        STAGE_SLOT = dict(
            dma="dma", mean="mean", mean_act="mean", tr="tr", tr_act="tr",
            vps="vps", vs="vps", vb="vb", vb_act="vb", prod="prod",
            tree="prod", tail="tail", exp="tail", pooled="pooled",
            final="pooled",
        )
        FNS = dict(
            dma=st_dma, mean=st_mean_pe, mean_act=st_mean_act,
            tr=st_transpose_pe, tr_act=st_transpose_act, vps=st_vps_pe,
            vs=st_vs_dve, vb=st_vb_pe, vb_act=st_vb_act, tail=st_tail_dve,
            exp=st_exp_act, pooled=st_pooled_pe, final=st_final,
        )
        # compressed items chain within a slot: emit their stages in
        # dependency order AFTER all steady-state work of that slot
        CHAIN_POS = dict(
            dma=20, mean=21, mean_act=22, tr=23, tr_act=24, vps=25, vs=26,
            vb=27, vb_act=28, prod=29, tree=30, tail=31, exp=32, pooled=33,
            final=34,
        )
        events = []
        for i in range(NI):
            off = OFF_BY_ITEM.get(i)
            key = PRIO if off is None else CHAIN_POS
            off = off or OFF_STD
            for st in PRIO:
                events.append((i + off[STAGE_SLOT[st]], key[st], i, st))
        events.sort()
        prods = {}
        for slot, _, i, st in events:
            if st == "prod":
                prods[i] = st_products_dve(i)
            elif st == "tree":
                st_tree_pool(i, prods[i])
            else:
                FNS[st](i)

    nc.compile()
    return nc


def _get_nc():
    if "nc" not in _CACHE:
        _CACHE["nc"] = _build()
    return _CACHE["nc"]


def _prep_inputs(x, mask, Wq, bq, Wk):
    """Host-side: transposed bf16 layout with ones column + scaled masks."""
    B, N, d, H, Wd = x.shape
    BN = B * N
    xr = np.asarray(x, dtype=np.float32).reshape(BN, d, NCH, 128)
    mr = np.asarray(mask).reshape(BN, NCH, 128).astype(np.float32)
    mt = mr.transpose(0, 2, 1)  # (BN, 128p, NCH)

    xt = np.zeros((BN, 128, NCH, WS), dtype=ml_dtypes.bfloat16)
    # pre-masked values and masked ones column: masked tokens are all-zero
    # rows, so they contribute nothing to pooled sums or Z
    xt[..., 0:d] = xr.transpose(0, 3, 2, 1) * mt[..., None]
    xt[..., d] = mt
    xt = np.ascontiguousarray(xt.reshape(BN, 128, NCH * WS))

    cnt = np.maximum(mr.reshape(BN, -1).sum(axis=1), 1.0)
    mprime = (mt / cnt[:, None, None]).astype(ml_dtypes.bfloat16)

    wqc = np.ascontiguousarray(np.asarray(Wq, dtype=np.float32))
    wkc = np.ascontiguousarray(np.asarray(Wk, dtype=np.float32))
    bq2 = np.ascontiguousarray(np.asarray(bq, dtype=np.float32).reshape(d, 1))

    in_maps = []
    for cid in range(NCORES):
        sl = slice(cid * NI, (cid + 1) * NI)
        mm = np.ascontiguousarray(
            mprime[sl].transpose(1, 0, 2).reshape(128, NI * NCH)
        )
        in_maps.append(
            {
                "xt": np.ascontiguousarray(xt[sl]),
                "mm": mm,
                "Wq": wqc,
                "Wk": wkc,
                "bq": bq2,
            }
        )
    return in_maps


def kernel(x, mask, Wq, bq, Wk, bk):
    from concourse.bass_utils import run_bass_kernel_spmd

    nc = _get_nc()
    B, N, d, H, Wd = x.shape
    in_maps = _prep_inputs(x, mask, Wq, bq, Wk)
    res = run_bass_kernel_spmd(nc, in_maps, core_ids=list(range(NCORES)))
    parts = [np.asarray(res.results[c]["out"]) for c in range(NCORES)]
    return np.concatenate(parts, axis=0).reshape(B, N, d).astype(np.float32)
